# revision 20
# baseline (speedup 1.0000x reference)
"""Trainium2 Bass kernel for GroundTruthBasedPriorNetwork.

Per-node tiny MLP over a banded DAG, batched over 131072 samples:
    x[b, n, p]  = gt_labels[b, parent_idx[n, p]]          (N=64 nodes, P=8)
    h[b, n, :]  = tanh(W1[n] @ x[b, n, :] + b1[n])        (HID=16)
    mus[b, n]   = W2[n] . h[b, n, :] + b2[n]
    logvars     = zeros

Pure data parallel over 8 NeuronCores (batch split 8x16384).  ScalarE
(tanh over 16.8M elems/core: 64 x (128,2048) tiles at (2048+352)/1.2
~= 1.97us each) is the roofline engine.  The steady-state phase period
is ~2.2us: the serial chain ACT -> L2 -> DVE-evac -> bank0-refill ->
next ACT exceeds the ACT slot by ~0.24us/phase because the Tile
framework's WAR tracking is effectively whole-tile (a refill of any
bank waits on the previous evacuation) and the PE stays at its 1.2 GHz
mid pstate for the whole kernel (46% duty never triggers 2.4 GHz;
LDWEIGHTS filler does not either).

The banded DAG means hidden block t (128 dims = nodes 8t..8t+8) only
reads input rows [8t-8, 8t+7) plus a bias row: K=16.  Four blocks are
row-tiled into PE row-groups 0/32/64/96 and run concurrently; the host
prepares band panels xb0/xb1 (quads t=0-3 / t=4-7, 128 partitions with
a ones row per 32-group) so one quad fills a persistent (128, 2048)
PSUM tile (2 quads = all 8 banks; the two quads ping-pong ACT vs PE).

Layer 2 per block needs only a (128, 8) stationary; four blocks are
col-tiled into col-groups 0/32/64/96, writing partition strips
32j..32j+8 of bank 0 of the SAME l1 quad-tile its Tanh just consumed.
One full-width DVE tensor_scalar_add(b2) evacuates the strips
(inactive lanes carry junk that the host discards).  Bank 0 of each
quad is refilled last (j order 1,2,3,0).

All PE/ACT/DVE instructions are chained with order-only dependencies
(add_dep_helper) pinning the software-pipelined emission order; the
Tile scheduler's cost model otherwise reorders the FIFO and causes
head-of-line blocking on the evacuate->refill edge.

Startup (~10.8us incl ~7us fixed runtime preamble): input DMA issues
cost the issuing sequencer ~0.6us each and transfers serialize per
queue, so w1b/w2c/b2 go on the scalar HWDGE queue, xb0 chunks on sync,
xb1 chunks on gpsimd (SWDGE), with the tanh table-load warmup issued
between them.  Outputs leave as two (128, BC) bf16 panels: full chunks
are DMA'd as four valid 8-partition strips (4x fewer HBM bytes); the
final chunk goes per-slab with the very last transfer on the
then-idle scalar queue so the tail (~6us) is not DMA-bound.
"""

import os

import numpy as np

NUM_NODES = 64
MAX_P = 8
HID = 16
HFULL = NUM_NODES * HID  # 1024
BATCH = 131072
NCORES = 8
BC = BATCH // NCORES  # 16384 per core
SLAB = 512
NSLAB = BC // SLAB  # 32
QW = 4 * SLAB  # 2048: quad tile width
OC = 4096  # output DMA chunk width
QUADS = ((0, 1, 2, 3), (4, 5, 6, 7))
ICHUNKS = [(0, 512), (512, 512), (1024, 1024), (2048, 2048),
           (4096, 4096), (8192, 8192)]

_COMPILED = {}


def _bf16(a):
    import ml_dtypes

    return np.asarray(a, np.float32).astype(ml_dtypes.bfloat16)


def _band_lo(t):
    return max(0, 8 * t - 8)


def _build_weights(W1, b1, W2, b2, parent_idx):
    """Host-side preprocessing of the tiny per-node weights."""
    W1 = np.asarray(W1, np.float32)
    b1 = np.asarray(b1, np.float32)
    W2 = np.asarray(W2, np.float32)
    b2 = np.asarray(b2, np.float32)
    parent_idx = np.asarray(parent_idx)

    # W1_full[j, 16n+h] = sum_p [parent_idx[n,p]==j] * W1[n,h,p]
    w1_full = np.zeros((NUM_NODES, HFULL), np.float32)
    for n in range(NUM_NODES):
        for p in range(MAX_P):
            j = int(parent_idx[n, p])
            w1_full[j, 16 * n : 16 * n + 16] += W1[n, :, p]

    # Row-tiled L1 stationaries: w1b[32j+i, 128q+c] = block t=4q+j's
    # weight for band row i (i=15 -> bias b1).
    w1b = np.zeros((128, 2 * 128), np.float32)
    for q, quad in enumerate(QUADS):
        for j, t in enumerate(quad):
            lo = _band_lo(t)
            nrow = 8 * t + 7 - lo if t > 0 else 7
            w1b[32 * j : 32 * j + nrow, 128 * q : 128 * (q + 1)] = \
                w1_full[lo : lo + nrow, 128 * t : 128 * (t + 1)]
            w1b[32 * j + 15, 128 * q : 128 * (q + 1)] = b1.reshape(HFULL)[
                128 * t : 128 * (t + 1)
            ]

    # Col-tiled L2 stationaries: w2c[p, 8t+k] = W2[8t+k, (128t+p)%16]
    # where (128t+p)//16 == 8t+k, else 0.
    w2c = np.zeros((128, NUM_NODES), np.float32)
    for t in range(8):
        for p in range(128):
            hf = 128 * t + p
            n = hf // HID
            w2c[p, n] = W2[n, hf % HID]

    wts = np.zeros((128, 2 * 128 + NUM_NODES), np.float32)
    wts[:, : 2 * 128] = w1b
    wts[:, 2 * 128 :] = w2c

    # b2 packed into evacuation strip layout: col q, partition 32j+i.
    b2r = np.zeros((128, 2), np.float32)
    for q, quad in enumerate(QUADS):
        for j, t in enumerate(quad):
            b2r[32 * j : 32 * j + 8, q] = b2[8 * t : 8 * t + 8]
    return _bf16(wts), np.ascontiguousarray(b2r)


def _build_bands(xc):
    """xc: (BC, 64) fp32 one core's batch. Returns 2 quad band panels."""
    xt = xc.T  # (64, BC)
    outs = []
    for quad in QUADS:
        xb = np.zeros((128, BC), np.float32)
        for j, t in enumerate(quad):
            lo = _band_lo(t)
            nrow = 8 * t + 7 - lo if t > 0 else 7
            xb[32 * j : 32 * j + nrow] = xt[lo : lo + nrow]
            xb[32 * j + 15] = 1.0
        outs.append(_bf16(xb))
    return outs


def _build_nc():
    import concourse.bacc as bacc
    import concourse.mybir as mybir
    import concourse.tile as tile
    from concourse.tile import add_dep_helper
    from contextlib import ExitStack

    f32 = mybir.dt.float32
    bf16 = mybir.dt.bfloat16

    nc = bacc.Bacc("TRN2", target_bir_lowering=False, debug=False,
                   num_devices=NCORES)

    CW = 2 * 128 + NUM_NODES  # 320
    xb_d = [
        nc.dram_tensor(f"xb{q}", [128, BC], bf16, kind="ExternalInput")
        for q in range(2)
    ]
    wts_d = nc.dram_tensor("wts", [128, CW], bf16, kind="ExternalInput")
    b2_d = nc.dram_tensor("b2", [128, 2], f32, kind="ExternalInput")
    out_d = [
        nc.dram_tensor(f"out{q}", [128, BC], bf16, kind="ExternalOutput")
        for q in range(2)
    ]

    last = {}  # per-engine previous instruction, for order pinning

    def pin(key, bi):
        if key in last:
            add_dep_helper(bi.ins, last[key].ins, sync=False,
                           reason="pipeline order")
        last[key] = bi
        return bi

    with tile.TileContext(nc) as tc, ExitStack() as ctx:
        consts = ctx.enter_context(tc.tile_pool(name="consts", bufs=1))
        xb_pool = ctx.enter_context(tc.tile_pool(name="xb", bufs=1))
        out_pool = ctx.enter_context(tc.tile_pool(name="outp", bufs=2))
        h_pool = ctx.enter_context(tc.tile_pool(name="h", bufs=6))
        l1_pool = ctx.enter_context(tc.tile_pool(name="l1", bufs=1, space="PSUM"))

        wts_sb = consts.tile([128, CW], bf16, tag="wts")
        b2_sb = consts.tile([128, 2], f32, tag="b2")
        dummy = consts.tile([128, 8], f32, tag="dummy")
        dummy2 = consts.tile([128, 8], bf16, tag="dummy2")

        # Pre-trigger the ACT tanh table load (~2.7us) while DMAs run;
        # the memset goes on GpSimd, the earliest-booting engine.
        pin("gp", nc.gpsimd.memset(dummy[:], 0.0))

        xb_sb = [
            xb_pool.tile([128, BC], bf16, tag=f"xb{q}", name=f"xb_sb{q}")
            for q in range(2)
        ]
        # Input DMA issues are spread over three queues (each dma_start
        # costs the issuing sequencer ~0.6us, and transfers serialize
        # within a queue): scalar -> all weights (separate queue so the
        # small weight transfers don't sit behind multi-MB xb chunks;
        # scalar is idle during the initial fill), sync -> xb0 chunks,
        # gpsimd -> xb1 chunks.
        w1b_sb = wts_sb[:, : 2 * 128]
        w2c_sb = wts_sb[:, 2 * 128 :]
        # w1b on the scalar HWDGE queue so its transfer overlaps xb0's
        # first chunk on sync; the tanh table-load warmup is issued
        # right after it.
        pin("act", nc.scalar.dma_start(wts_sb[:, : 2 * 128],
                                       wts_d.ap()[:, : 2 * 128]))
        pin("act", nc.scalar.activation(dummy2[:], dummy[:],
                                        mybir.ActivationFunctionType.Tanh))
        pin("act", nc.scalar.dma_start(wts_sb[:, 2 * 128 :],
                                       wts_d.ap()[:, 2 * 128 :]))
        pin("act", nc.scalar.dma_start(b2_sb[:], b2_d.ap()))
        for c0, w in ICHUNKS:
            pin("sync", nc.sync.dma_start(xb_sb[0][:, c0 : c0 + w],
                                          xb_d[0].ap()[:, c0 : c0 + w]))
            pin("gp", nc.gpsimd.dma_start(xb_sb[1][:, c0 : c0 + w],
                                          xb_d[1].ap()[:, c0 : c0 + w]))

        # Persistent PSUM quad tiles (no pool rotation): subtile dep
        # tracking then gives bank-granular WAR edges, so only the j=0
        # (bank0) refill waits on the mus evacuation instead of the
        # whole-quad refill waiting on it (slot-reuse WAR was the
        # baseline's steady-state pacer).
        l1q = [
            l1_pool.tile([128, QW], f32, tag=f"l1t{q}", name=f"l1q{q}")
            for q in range(2)
        ]

        out_tiles = {}
        out_fill = {}

        def out_tile(q, k):
            if (q, k) not in out_tiles:
                out_tiles[(q, k)] = out_pool.tile(
                    [128, OC], bf16, tag=f"oq{q}", name=f"out_q{q}_k{k}"
                )
                out_fill[(q, k)] = 0
            return out_tiles[(q, k)]

        h_live = {}

        def emit_l2_mm(s, q):
            h = h_live.pop((s, q))
            l1 = l1q[q]
            for j, t in enumerate(QUADS[q]):
                pin("pe", nc.tensor.matmul(
                    l1[32 * j : 32 * j + 8, 0:SLAB],
                    w2c_sb[:, 8 * t : 8 * t + 8],
                    h[:, SLAB * j : SLAB * (j + 1)],
                    start=True,
                    stop=True,
                    tile_position=(0, 32 * j),
                    skip_group_check=True,
                ))

        def emit_evac(s, q):
            l1 = l1q[q]
            k, oo = divmod(s * SLAB, OC)
            ot = out_tile(q, k)
            if s == NSLAB - 1:
                # Final slab: split the evacuation in halves and DMA
                # each half as soon as it lands (on the two idle HWDGE
                # queues -- all ACTIVATEs and earlier DMAs have retired)
                # so the last transfer overlaps the second half-evac
                # instead of serializing behind one full evac.
                qk = ("sync", nc.sync) if q == 0 else ("act", nc.scalar)
                half = SLAB // 2
                for p in range(2):
                    pin("dve", nc.vector.tensor_scalar_add(
                        ot[:, oo + p * half : oo + (p + 1) * half],
                        l1[:, p * half : (p + 1) * half],
                        b2_sb[:, q : q + 1],
                    ))
                    pin(qk[0], qk[1].dma_start(
                        out_d[q].ap()[:, s * SLAB + p * half
                                      : s * SLAB + (p + 1) * half],
                        ot[:, oo + p * half : oo + (p + 1) * half],
                    ))
                out_fill[(q, k)] += 1
                del out_tiles[(q, k)]
                return
            pin("dve", nc.vector.tensor_scalar_add(
                ot[:, oo : oo + SLAB], l1[:, 0:SLAB], b2_sb[:, q : q + 1]
            ))
            out_fill[(q, k)] += 1
            if k == BC // OC - 1:
                # Final chunk: DMA per slab so the kernel tail only
                # waits on one small last transfer.  gpsimd's SWDGE
                # adds ~1us per issue and a multi-us tail drain, so the
                # last slabs stay off it.
                lo = (out_fill[(q, k)] - 1) * SLAB
                qk = ("sync", nc.sync) if q == 0 else ("gp", nc.gpsimd)
                pin(qk[0], qk[1].dma_start(
                    out_d[q].ap()[:, k * OC + lo : k * OC + lo + SLAB],
                    ot[:, lo : lo + SLAB],
                ))
            elif out_fill[(q, k)] == OC // SLAB:
                # Only partitions 32j..32j+8 carry mus; DMA just those
                # strips (4x fewer HBM bytes than the full panel).
                qk = ("sync", nc.sync) if q == 0 else ("gp", nc.gpsimd)
                for j in range(4):
                    pin(qk[0], qk[1].dma_start(
                        out_d[q].ap()[32 * j : 32 * j + 8,
                                      k * OC : (k + 1) * OC],
                        ot[32 * j : 32 * j + 8, :],
                    ))
                del out_tiles[(q, k)]

        def l1_fill(s, q, j):
            c = s * SLAB
            pin("pe", nc.tensor.matmul(
                l1q[q][:, SLAB * j : SLAB * (j + 1)],
                w1b_sb[32 * j : 32 * j + 16, 128 * q : 128 * (q + 1)],
                xb_sb[q][32 * j : 32 * j + 16, c : c + SLAB],
                start=True,
                stop=True,
                tile_position=(32 * j, 0),
            ))

        for s in range(NSLAB):
            for q in range(2):
                # Per-phase order: L2(s-1), evac(s-1), refill j=1..3,0,
                # tanh.  Dep tracking is whole-tile, so the refill waits
                # on the evac regardless of j order; the serial chain
                # ACT->L2->evac->refill paces the pipeline at the cold
                # 1.2 GHz PE clock.  The dummy LDWEIGHTS below keep the
                # PE activity window hot so DVFS lifts PE to 2.4 GHz,
                # shrinking the chain below the ACT slot.
                if s > 0:
                    emit_l2_mm(s - 1, q)
                    emit_evac(s - 1, q)
                for j in (1, 2, 3, 0):
                    l1_fill(s, q, j)
                h = h_pool.tile([128, QW], bf16, tag="h")
                pin("act", nc.scalar.activation(
                    h[:], l1q[q][:], mybir.ActivationFunctionType.Tanh))
                h_live[(s, q)] = h
                nspam = 2 if s < 2 else 6
                for _ in range(nspam):
                    pin("pe", nc.tensor.ldweights(
                        w1b_sb[0:16, 0:128], tile_position=(0, 0)))
        for q in range(2):
            emit_l2_mm(NSLAB - 1, q)
            emit_evac(NSLAB - 1, q)

    nc.finalize()
    return nc


def _get_nc():
    if "nc" not in _COMPILED:
        _COMPILED["nc"] = _build_nc()
    return _COMPILED["nc"]


def kernel(gt_labels, W1, b1, W2, b2, parent_idx):
    from concourse.bass_utils import run_bass_kernel_spmd

    gt_labels = np.asarray(gt_labels, np.float32)
    wts, b2r = _build_weights(W1, b1, W2, b2, parent_idx)

    in_maps = []
    for c in range(NCORES):
        xb = _build_bands(gt_labels[c * BC : (c + 1) * BC])
        in_maps.append({"xb0": xb[0], "xb1": xb[1], "wts": wts, "b2": b2r})

    nc = _get_nc()
    trace = bool(int(os.environ.get("KERNEL_TRACE", "0")))
    res = run_bass_kernel_spmd(nc, in_maps, list(range(NCORES)), trace=trace)
    if trace and res.exec_time_ns is not None:
        print(f"HW exec time: {res.exec_time_ns} ns")
        _COMPILED["exec_time_ns"] = res.exec_time_ns

    mus = np.empty((BATCH, NUM_NODES), np.float32)
    for c in range(NCORES):
        rows = []
        for q in range(2):
            panel = np.asarray(res.results[c][f"out{q}"], np.float32)
            for j in range(4):
                rows.append(panel[32 * j : 32 * j + 8])  # nodes 32q+8j..+8
        mus[c * BC : (c + 1) * BC] = np.concatenate(rows, axis=0).T
    mus = mus.reshape(BATCH, NUM_NODES, 1)
    logvars = np.zeros_like(mus)
    return mus, logvars



# revision 23
# speedup vs baseline: 1.0164x; 1.0164x over previous
"""Trainium2 Bass kernel for GroundTruthBasedPriorNetwork.

Per-node tiny MLP over a banded DAG, batched over 131072 samples:
    x[b, n, p]  = gt_labels[b, parent_idx[n, p]]          (N=64 nodes, P=8)
    h[b, n, :]  = tanh(W1[n] @ x[b, n, :] + b1[n])        (HID=16)
    mus[b, n]   = W2[n] . h[b, n, :] + b2[n]
    logvars     = zeros

Pure data parallel over 8 NeuronCores (batch split 8x16384).  ScalarE
(tanh over 16.8M elems/core: 64 x (128,2048) tiles at (2048+352)/1.2
~= 1.97us each) is the roofline engine.  The steady-state phase period
is ~2.2us: the serial chain ACT -> L2 -> DVE-evac -> bank0-refill ->
next ACT exceeds the ACT slot by ~0.24us/phase because the Tile
framework's WAR tracking is effectively whole-tile (a refill of any
bank waits on the previous evacuation) and the PE stays at its 1.2 GHz
mid pstate for the whole kernel (46% duty never triggers 2.4 GHz;
LDWEIGHTS filler does not either).

The banded DAG means hidden block t (128 dims = nodes 8t..8t+8) only
reads input rows [8t-8, 8t+7) plus a bias row: K=16.  Four blocks are
row-tiled into PE row-groups 0/32/64/96 and run concurrently; the host
prepares band panels xb0/xb1 (quads t=0-3 / t=4-7, 128 partitions with
a ones row per 32-group) so one quad fills a persistent (128, 2048)
PSUM tile (2 quads = all 8 banks; the two quads ping-pong ACT vs PE).

Layer 2 per block needs only a (128, 8) stationary; four blocks are
col-tiled into col-groups 0/32/64/96, writing partition strips
32j..32j+8 of bank 0 of the SAME l1 quad-tile its Tanh just consumed.
One full-width DVE tensor_scalar_add(b2) evacuates the strips
(inactive lanes carry junk that the host discards).  Bank 0 of each
quad is refilled last (j order 1,2,3,0).

All PE/ACT/DVE instructions are chained with order-only dependencies
(add_dep_helper) pinning the software-pipelined emission order; the
Tile scheduler's cost model otherwise reorders the FIFO and causes
head-of-line blocking on the evacuate->refill edge.

Startup (~10.8us incl ~7us fixed runtime preamble): input DMA issues
cost the issuing sequencer ~0.6us each and transfers serialize per
queue, so w1b/w2c/b2 go on the scalar HWDGE queue, xb0 chunks on sync,
xb1 chunks on gpsimd (SWDGE), with the tanh table-load warmup issued
between them.  Outputs leave as two (128, BC) bf16 panels: full chunks
are DMA'd as four valid 8-partition strips (4x fewer HBM bytes); the
final chunk goes per-slab with the very last transfer on the
then-idle scalar queue so the tail (~6us) is not DMA-bound.
"""

import os

import numpy as np

NUM_NODES = 64
MAX_P = 8
HID = 16
HFULL = NUM_NODES * HID  # 1024
BATCH = 131072
NCORES = 8
BC = BATCH // NCORES  # 16384 per core
SLAB = 512
NSLAB = BC // SLAB  # 32
QW = 4 * SLAB  # 2048: quad tile width
OC = 4096  # output DMA chunk width
QUADS = ((0, 1, 2, 3), (4, 5, 6, 7))
ICHUNKS = [(0, 512), (512, 512), (1024, 1024), (2048, 2048),
           (4096, 4096), (8192, 8192)]

_COMPILED = {}


def _bf16(a):
    import ml_dtypes

    return np.asarray(a, np.float32).astype(ml_dtypes.bfloat16)


def _band_lo(t):
    return max(0, 8 * t - 8)


def _build_weights(W1, b1, W2, b2, parent_idx):
    """Host-side preprocessing of the tiny per-node weights."""
    W1 = np.asarray(W1, np.float32)
    b1 = np.asarray(b1, np.float32)
    W2 = np.asarray(W2, np.float32)
    b2 = np.asarray(b2, np.float32)
    parent_idx = np.asarray(parent_idx)

    # W1_full[j, 16n+h] = sum_p [parent_idx[n,p]==j] * W1[n,h,p]
    w1_full = np.zeros((NUM_NODES, HFULL), np.float32)
    for n in range(NUM_NODES):
        for p in range(MAX_P):
            j = int(parent_idx[n, p])
            w1_full[j, 16 * n : 16 * n + 16] += W1[n, :, p]

    # Row-tiled L1 stationaries: w1b[32j+i, 128q+c] = block t=4q+j's
    # weight for band row i (i=15 -> bias b1).
    w1b = np.zeros((128, 2 * 128), np.float32)
    for q, quad in enumerate(QUADS):
        for j, t in enumerate(quad):
            lo = _band_lo(t)
            nrow = 8 * t + 7 - lo if t > 0 else 7
            w1b[32 * j : 32 * j + nrow, 128 * q : 128 * (q + 1)] = \
                w1_full[lo : lo + nrow, 128 * t : 128 * (t + 1)]
            w1b[32 * j + 15, 128 * q : 128 * (q + 1)] = b1.reshape(HFULL)[
                128 * t : 128 * (t + 1)
            ]

    # Col-tiled L2 stationaries: w2c[p, 8t+k] = W2[8t+k, (128t+p)%16]
    # where (128t+p)//16 == 8t+k, else 0.
    w2c = np.zeros((128, NUM_NODES), np.float32)
    for t in range(8):
        for p in range(128):
            hf = 128 * t + p
            n = hf // HID
            w2c[p, n] = W2[n, hf % HID]


    # b2 packed into evacuation strip layout: col q, partition 32j+i.
    b2r = np.zeros((128, 2), np.float32)
    for q, quad in enumerate(QUADS):
        for j, t in enumerate(quad):
            b2r[32 * j : 32 * j + 8, q] = b2[8 * t : 8 * t + 8]
    return _bf16(w1b), _bf16(w2c), np.ascontiguousarray(b2r)


def _build_bands(xc):
    """xc: (BC, 64) fp32 one core's batch. Returns 2 quad band panels."""
    xt = xc.T  # (64, BC)
    outs = []
    for quad in QUADS:
        xb = np.zeros((128, BC), np.float32)
        for j, t in enumerate(quad):
            lo = _band_lo(t)
            nrow = 8 * t + 7 - lo if t > 0 else 7
            xb[32 * j : 32 * j + nrow] = xt[lo : lo + nrow]
            xb[32 * j + 15] = 1.0
        outs.append(_bf16(xb))
    return outs


def _build_nc():
    import concourse.bacc as bacc
    import concourse.mybir as mybir
    import concourse.tile as tile
    from concourse.tile import add_dep_helper
    from contextlib import ExitStack

    f32 = mybir.dt.float32
    bf16 = mybir.dt.bfloat16

    nc = bacc.Bacc("TRN2", target_bir_lowering=False, debug=False,
                   num_devices=NCORES)

    CW = 2 * 128 + NUM_NODES  # 320
    xb_d = [
        nc.dram_tensor(f"xb{q}", [128, BC], bf16, kind="ExternalInput")
        for q in range(2)
    ]
    w1b_d = nc.dram_tensor("w1b", [128, 2 * 128], bf16, kind="ExternalInput")
    w2c_d = nc.dram_tensor("w2c", [128, NUM_NODES], bf16, kind="ExternalInput")
    b2_d = nc.dram_tensor("b2", [128, 2], f32, kind="ExternalInput")
    out_d = [
        nc.dram_tensor(f"out{q}", [128, BC], bf16, kind="ExternalOutput")
        for q in range(2)
    ]

    last = {}  # per-engine previous instruction, for order pinning

    def pin(key, bi):
        if key in last:
            add_dep_helper(bi.ins, last[key].ins, sync=False,
                           reason="pipeline order")
        last[key] = bi
        return bi

    with tile.TileContext(nc) as tc, ExitStack() as ctx:
        consts = ctx.enter_context(tc.tile_pool(name="consts", bufs=1))
        xb_pool = ctx.enter_context(tc.tile_pool(name="xb", bufs=1))
        out_pool = ctx.enter_context(tc.tile_pool(name="outp", bufs=2))
        h_pool = ctx.enter_context(tc.tile_pool(name="h", bufs=6))
        l1_pool = ctx.enter_context(tc.tile_pool(name="l1", bufs=1, space="PSUM"))

        w1b_sb = consts.tile([128, 2 * 128], bf16, tag="w1b")
        w2c_sb = consts.tile([128, NUM_NODES], bf16, tag="w2c")
        b2_sb = consts.tile([128, 2], f32, tag="b2")
        dummy = consts.tile([128, 8], f32, tag="dummy")
        dummy2 = consts.tile([128, 8], bf16, tag="dummy2")

        # Pre-trigger the ACT tanh table load (~2.7us) while DMAs run;
        # the memset goes on GpSimd, the earliest-booting engine.
        pin("gp", nc.gpsimd.memset(dummy[:], 0.0))

        xb_sb = [
            xb_pool.tile([128, BC], bf16, tag=f"xb{q}", name=f"xb_sb{q}")
            for q in range(2)
        ]
        # Input DMA issues are spread over three queues (each dma_start
        # costs the issuing sequencer ~0.6us, and transfers serialize
        # within a queue): scalar -> all weights (separate queue so the
        # small weight transfers don't sit behind multi-MB xb chunks;
        # scalar is idle during the initial fill), sync -> xb0 chunks,
        # gpsimd -> xb1 chunks.
        # w1b on the scalar HWDGE queue so its transfer overlaps xb0's
        # first chunk on sync; the tanh table-load warmup is issued
        # right after it.  w1b/w2c are separate tensors+tiles so the
        # first L1 LDWEIGHTS never waits on the w2c transfer.
        pin("act", nc.scalar.dma_start(w1b_sb[:], w1b_d.ap()))
        pin("act", nc.scalar.activation(dummy2[:], dummy[:],
                                        mybir.ActivationFunctionType.Tanh))
        pin("act", nc.scalar.dma_start(w2c_sb[:], w2c_d.ap()))
        pin("act", nc.scalar.dma_start(b2_sb[:], b2_d.ap()))
        for c0, w in ICHUNKS:
            pin("sync", nc.sync.dma_start(xb_sb[0][:, c0 : c0 + w],
                                          xb_d[0].ap()[:, c0 : c0 + w]))
            pin("gp", nc.gpsimd.dma_start(xb_sb[1][:, c0 : c0 + w],
                                          xb_d[1].ap()[:, c0 : c0 + w]))

        # Persistent PSUM quad tiles (no pool rotation): subtile dep
        # tracking then gives bank-granular WAR edges, so only the j=0
        # (bank0) refill waits on the mus evacuation instead of the
        # whole-quad refill waiting on it (slot-reuse WAR was the
        # baseline's steady-state pacer).
        l1q = [
            l1_pool.tile([128, QW], f32, tag=f"l1t{q}", name=f"l1q{q}")
            for q in range(2)
        ]

        out_tiles = {}
        out_fill = {}

        def out_tile(q, k):
            if (q, k) not in out_tiles:
                out_tiles[(q, k)] = out_pool.tile(
                    [128, OC], bf16, tag=f"oq{q}", name=f"out_q{q}_k{k}"
                )
                out_fill[(q, k)] = 0
            return out_tiles[(q, k)]

        h_live = {}

        def emit_l2_mm(s, q):
            h = h_live.pop((s, q))
            l1 = l1q[q]
            for j, t in enumerate(QUADS[q]):
                pin("pe", nc.tensor.matmul(
                    l1[32 * j : 32 * j + 8, 0:SLAB],
                    w2c_sb[:, 8 * t : 8 * t + 8],
                    h[:, SLAB * j : SLAB * (j + 1)],
                    start=True,
                    stop=True,
                    tile_position=(0, 32 * j),
                    skip_group_check=True,
                ))

        def emit_evac(s, q):
            l1 = l1q[q]
            k, oo = divmod(s * SLAB, OC)
            ot = out_tile(q, k)
            pin("dve", nc.vector.tensor_scalar_add(
                ot[:, oo : oo + SLAB], l1[:, 0:SLAB], b2_sb[:, q : q + 1]
            ))
            out_fill[(q, k)] += 1
            if k == BC // OC - 1:
                # Final chunk: DMA per slab so the kernel tail only
                # waits on one small last transfer.  The very last DMA
                # (all ACTIVATEs retired by then) goes on the idle
                # scalar HWDGE queue -- gpsimd's SWDGE adds ~1us per
                # issue and a multi-us tail drain.
                lo = (out_fill[(q, k)] - 1) * SLAB
                if s == NSLAB - 1 and q == 1:
                    qk = ("act", nc.scalar)
                elif q == 0:
                    qk = ("sync", nc.sync)
                else:
                    qk = ("gp", nc.gpsimd)
                pin(qk[0], qk[1].dma_start(
                    out_d[q].ap()[:, k * OC + lo : k * OC + lo + SLAB],
                    ot[:, lo : lo + SLAB],
                ))
                if out_fill[(q, k)] == OC // SLAB:
                    del out_tiles[(q, k)]
            elif out_fill[(q, k)] == OC // SLAB:
                # Only partitions 32j..32j+8 carry mus; DMA just those
                # strips (4x fewer HBM bytes than the full panel).
                qk = ("sync", nc.sync) if q == 0 else ("gp", nc.gpsimd)
                for j in range(4):
                    pin(qk[0], qk[1].dma_start(
                        out_d[q].ap()[32 * j : 32 * j + 8,
                                      k * OC : (k + 1) * OC],
                        ot[32 * j : 32 * j + 8, :],
                    ))
                del out_tiles[(q, k)]

        def l1_fill(s, q, j):
            c = s * SLAB
            pin("pe", nc.tensor.matmul(
                l1q[q][:, SLAB * j : SLAB * (j + 1)],
                w1b_sb[32 * j : 32 * j + 16, 128 * q : 128 * (q + 1)],
                xb_sb[q][32 * j : 32 * j + 16, c : c + SLAB],
                start=True,
                stop=True,
                tile_position=(32 * j, 0),
            ))

        for s in range(NSLAB):
            for q in range(2):
                # Per-phase order: L2(s-1), evac(s-1), refill j=1..3,0,
                # tanh.  Dep tracking is whole-tile, so the refill waits
                # on the evac regardless of j order; the serial chain
                # ACT->L2->evac->refill paces the pipeline at the cold
                # 1.2 GHz PE clock.  The dummy LDWEIGHTS below keep the
                # PE activity window hot so DVFS lifts PE to 2.4 GHz,
                # shrinking the chain below the ACT slot.
                if s > 0:
                    emit_l2_mm(s - 1, q)
                    emit_evac(s - 1, q)
                for j in (1, 2, 3, 0):
                    l1_fill(s, q, j)
                h = h_pool.tile([128, QW], bf16, tag="h")
                pin("act", nc.scalar.activation(
                    h[:], l1q[q][:], mybir.ActivationFunctionType.Tanh))
                h_live[(s, q)] = h
                nspam = 2 if s < 2 else 6
                for _ in range(nspam):
                    pin("pe", nc.tensor.ldweights(
                        w1b_sb[0:16, 0:128], tile_position=(0, 0)))
        for q in range(2):
            emit_l2_mm(NSLAB - 1, q)
            emit_evac(NSLAB - 1, q)

    nc.finalize()
    return nc


def _get_nc():
    if "nc" not in _COMPILED:
        _COMPILED["nc"] = _build_nc()
    return _COMPILED["nc"]


def kernel(gt_labels, W1, b1, W2, b2, parent_idx):
    from concourse.bass_utils import run_bass_kernel_spmd

    gt_labels = np.asarray(gt_labels, np.float32)
    w1b_h, w2c_h, b2r = _build_weights(W1, b1, W2, b2, parent_idx)

    in_maps = []
    for c in range(NCORES):
        xb = _build_bands(gt_labels[c * BC : (c + 1) * BC])
        in_maps.append({"xb0": xb[0], "xb1": xb[1], "w1b": w1b_h,
                        "w2c": w2c_h, "b2": b2r})

    nc = _get_nc()
    trace = bool(int(os.environ.get("KERNEL_TRACE", "0")))
    res = run_bass_kernel_spmd(nc, in_maps, list(range(NCORES)), trace=trace)
    if trace and res.exec_time_ns is not None:
        print(f"HW exec time: {res.exec_time_ns} ns")
        _COMPILED["exec_time_ns"] = res.exec_time_ns

    mus = np.empty((BATCH, NUM_NODES), np.float32)
    for c in range(NCORES):
        rows = []
        for q in range(2):
            panel = np.asarray(res.results[c][f"out{q}"], np.float32)
            for j in range(4):
                rows.append(panel[32 * j : 32 * j + 8])  # nodes 32q+8j..+8
        mus[c * BC : (c + 1) * BC] = np.concatenate(rows, axis=0).T
    mus = mus.reshape(BATCH, NUM_NODES, 1)
    logvars = np.zeros_like(mus)
    return mus, logvars



# revision 24
# speedup vs baseline: 1.0239x; 1.0074x over previous
"""Trainium2 Bass kernel for GroundTruthBasedPriorNetwork.

Per-node tiny MLP over a banded DAG, batched over 131072 samples:
    x[b, n, p]  = gt_labels[b, parent_idx[n, p]]          (N=64 nodes, P=8)
    h[b, n, :]  = tanh(W1[n] @ x[b, n, :] + b1[n])        (HID=16)
    mus[b, n]   = W2[n] . h[b, n, :] + b2[n]
    logvars     = zeros

Pure data parallel over 8 NeuronCores (batch split 8x16384).  ScalarE
(tanh over 16.8M elems/core: 64 x (128,2048) tiles at (2048+352)/1.2
~= 1.97us each) is the roofline engine.  The steady-state phase period
is ~2.2us: the serial chain ACT -> L2 -> DVE-evac -> bank0-refill ->
next ACT exceeds the ACT slot by ~0.24us/phase because the Tile
framework's WAR tracking is effectively whole-tile (a refill of any
bank waits on the previous evacuation) and the PE stays at its 1.2 GHz
mid pstate for the whole kernel (46% duty never triggers 2.4 GHz;
LDWEIGHTS filler does not either).

The banded DAG means hidden block t (128 dims = nodes 8t..8t+8) only
reads input rows [8t-8, 8t+7) plus a bias row: K=16.  Four blocks are
row-tiled into PE row-groups 0/32/64/96 and run concurrently; the host
prepares band panels xb0/xb1 (quads t=0-3 / t=4-7, 128 partitions with
a ones row per 32-group) so one quad fills a persistent (128, 2048)
PSUM tile (2 quads = all 8 banks; the two quads ping-pong ACT vs PE).

Layer 2 per block needs only a (128, 8) stationary; four blocks are
col-tiled into col-groups 0/32/64/96, writing partition strips
32j..32j+8 of bank 0 of the SAME l1 quad-tile its Tanh just consumed.
One full-width DVE tensor_scalar_add(b2) evacuates the strips
(inactive lanes carry junk that the host discards).  Bank 0 of each
quad is refilled last (j order 1,2,3,0).

All PE/ACT/DVE instructions are chained with order-only dependencies
(add_dep_helper) pinning the software-pipelined emission order; the
Tile scheduler's cost model otherwise reorders the FIFO and causes
head-of-line blocking on the evacuate->refill edge.

The emission order per phase [L2, evac, j1..j3, j0] is optimal for
this scheduler: it enforces its linearized cross-engine order at
run granularity, so any emission that places the j refills between
L2 and the evac serializes ~0.6us of extra PE work into the chain
(measured: 188us vs 159us).

Startup (~11us incl ~7us fixed runtime preamble): input DMA issues
cost the issuing sequencer ~0.6us each and transfers serialize per
queue, so w1b/w2c/b2 go on the scalar HWDGE queue (w1b and w2c as
separate tensors so the first LDWEIGHTS cannot alias-wait on w2c),
xb0 chunks on sync, xb1 chunks on gpsimd (SWDGE), with the tanh
table-load warmup issued between them.  Outputs leave as two
(128, BC) bf16 panels: full chunks are DMA'd as four valid
8-partition strips (4x fewer HBM bytes); the final chunk goes
per-slab with the very last transfer on the then-idle scalar queue
so the tail (~6us) is not DMA-bound.
"""

import os

import numpy as np

NUM_NODES = 64
MAX_P = 8
HID = 16
HFULL = NUM_NODES * HID  # 1024
BATCH = 131072
NCORES = 8
BC = BATCH // NCORES  # 16384 per core
SLAB = 512
NSLAB = BC // SLAB  # 32
QW = 4 * SLAB  # 2048: quad tile width
OC = 4096  # output DMA chunk width
QUADS = ((0, 1, 2, 3), (4, 5, 6, 7))
ICHUNKS = [(0, 512), (512, 512), (1024, 1024), (2048, 2048),
           (4096, 4096), (8192, 8192)]

_COMPILED = {}


def _bf16(a):
    import ml_dtypes

    return np.asarray(a, np.float32).astype(ml_dtypes.bfloat16)


def _band_lo(t):
    return max(0, 8 * t - 8)


def _build_weights(W1, b1, W2, b2, parent_idx):
    """Host-side preprocessing of the tiny per-node weights."""
    W1 = np.asarray(W1, np.float32)
    b1 = np.asarray(b1, np.float32)
    W2 = np.asarray(W2, np.float32)
    b2 = np.asarray(b2, np.float32)
    parent_idx = np.asarray(parent_idx)

    # W1_full[j, 16n+h] = sum_p [parent_idx[n,p]==j] * W1[n,h,p]
    w1_full = np.zeros((NUM_NODES, HFULL), np.float32)
    for n in range(NUM_NODES):
        for p in range(MAX_P):
            j = int(parent_idx[n, p])
            w1_full[j, 16 * n : 16 * n + 16] += W1[n, :, p]

    # Row-tiled L1 stationaries: w1b[32j+i, 128q+c] = block t=4q+j's
    # weight for band row i (i=15 -> bias b1).
    w1b = np.zeros((128, 2 * 128), np.float32)
    for q, quad in enumerate(QUADS):
        for j, t in enumerate(quad):
            lo = _band_lo(t)
            nrow = 8 * t + 7 - lo if t > 0 else 7
            w1b[32 * j : 32 * j + nrow, 128 * q : 128 * (q + 1)] = \
                w1_full[lo : lo + nrow, 128 * t : 128 * (t + 1)]
            w1b[32 * j + 15, 128 * q : 128 * (q + 1)] = b1.reshape(HFULL)[
                128 * t : 128 * (t + 1)
            ]

    # Col-tiled L2 stationaries: w2c[p, 8t+k] = W2[8t+k, (128t+p)%16]
    # where (128t+p)//16 == 8t+k, else 0.
    w2c = np.zeros((128, NUM_NODES), np.float32)
    for t in range(8):
        for p in range(128):
            hf = 128 * t + p
            n = hf // HID
            w2c[p, n] = W2[n, hf % HID]


    # b2 packed into evacuation strip layout: col q, partition 32j+i.
    b2r = np.zeros((128, 2), np.float32)
    for q, quad in enumerate(QUADS):
        for j, t in enumerate(quad):
            b2r[32 * j : 32 * j + 8, q] = b2[8 * t : 8 * t + 8]
    return _bf16(w1b), _bf16(w2c), np.ascontiguousarray(b2r)


def _build_bands(xc):
    """xc: (BC, 64) fp32 one core's batch. Returns 2 quad band panels."""
    xt = xc.T  # (64, BC)
    outs = []
    for quad in QUADS:
        xb = np.zeros((128, BC), np.float32)
        for j, t in enumerate(quad):
            lo = _band_lo(t)
            nrow = 8 * t + 7 - lo if t > 0 else 7
            xb[32 * j : 32 * j + nrow] = xt[lo : lo + nrow]
            xb[32 * j + 15] = 1.0
        outs.append(_bf16(xb))
    return outs


def _build_nc():
    import concourse.bacc as bacc
    import concourse.mybir as mybir
    import concourse.tile as tile
    from concourse.tile import add_dep_helper
    from contextlib import ExitStack

    f32 = mybir.dt.float32
    bf16 = mybir.dt.bfloat16

    nc = bacc.Bacc("TRN2", target_bir_lowering=False, debug=False,
                   num_devices=NCORES)

    CW = 2 * 128 + NUM_NODES  # 320
    xb_d = [
        nc.dram_tensor(f"xb{q}", [128, BC], bf16, kind="ExternalInput")
        for q in range(2)
    ]
    w1b_d = nc.dram_tensor("w1b", [128, 2 * 128], bf16, kind="ExternalInput")
    w2c_d = nc.dram_tensor("w2c", [128, NUM_NODES], bf16, kind="ExternalInput")
    b2_d = nc.dram_tensor("b2", [128, 2], f32, kind="ExternalInput")
    out_d = [
        nc.dram_tensor(f"out{q}", [128, BC], bf16, kind="ExternalOutput")
        for q in range(2)
    ]

    last = {}  # per-engine previous instruction, for order pinning

    def pin(key, bi):
        if key in last:
            add_dep_helper(bi.ins, last[key].ins, sync=False,
                           reason="pipeline order")
        last[key] = bi
        return bi

    with tile.TileContext(nc) as tc, ExitStack() as ctx:
        consts = ctx.enter_context(tc.tile_pool(name="consts", bufs=1))
        xb_pool = ctx.enter_context(tc.tile_pool(name="xb", bufs=1))
        out_pool = ctx.enter_context(tc.tile_pool(name="outp", bufs=2))
        h_pool = ctx.enter_context(tc.tile_pool(name="h", bufs=6))
        l1_pool = ctx.enter_context(tc.tile_pool(name="l1", bufs=1, space="PSUM"))

        w1b_sb = consts.tile([128, 2 * 128], bf16, tag="w1b")
        w2c_sb = consts.tile([128, NUM_NODES], bf16, tag="w2c")
        b2_sb = consts.tile([128, 2], f32, tag="b2")
        dummy = consts.tile([128, 8], f32, tag="dummy")
        dummy2 = consts.tile([128, 8], bf16, tag="dummy2")

        # Pre-trigger the ACT tanh table load (~2.7us) while DMAs run;
        # the memset goes on GpSimd, the earliest-booting engine.
        pin("gp", nc.gpsimd.memset(dummy[:], 0.0))

        xb_sb = [
            xb_pool.tile([128, BC], bf16, tag=f"xb{q}", name=f"xb_sb{q}")
            for q in range(2)
        ]
        # Input DMA issues are spread over three queues (each dma_start
        # costs the issuing sequencer ~0.6us, and transfers serialize
        # within a queue): scalar -> all weights (separate queue so the
        # small weight transfers don't sit behind multi-MB xb chunks;
        # scalar is idle during the initial fill), sync -> xb0 chunks,
        # gpsimd -> xb1 chunks.
        # w1b on the scalar HWDGE queue so its transfer overlaps xb0's
        # first chunk on sync; the tanh table-load warmup is issued
        # right after it.  w1b/w2c are separate tensors+tiles so the
        # first L1 LDWEIGHTS never waits on the w2c transfer.
        pin("act", nc.scalar.dma_start(w1b_sb[:], w1b_d.ap()))
        pin("act", nc.scalar.activation(dummy2[:], dummy[:],
                                        mybir.ActivationFunctionType.Tanh))
        pin("act", nc.scalar.dma_start(w2c_sb[:], w2c_d.ap()))
        pin("act", nc.scalar.dma_start(b2_sb[:], b2_d.ap()))
        for c0, w in ICHUNKS:
            pin("sync", nc.sync.dma_start(xb_sb[0][:, c0 : c0 + w],
                                          xb_d[0].ap()[:, c0 : c0 + w]))
            pin("gp", nc.gpsimd.dma_start(xb_sb[1][:, c0 : c0 + w],
                                          xb_d[1].ap()[:, c0 : c0 + w]))

        # Persistent PSUM quad tiles (no pool rotation): subtile dep
        # tracking then gives bank-granular WAR edges, so only the j=0
        # (bank0) refill waits on the mus evacuation instead of the
        # whole-quad refill waiting on it (slot-reuse WAR was the
        # baseline's steady-state pacer).
        l1q = [
            l1_pool.tile([128, QW], f32, tag=f"l1t{q}", name=f"l1q{q}")
            for q in range(2)
        ]

        out_tiles = {}
        out_fill = {}

        def out_tile(q, k):
            if (q, k) not in out_tiles:
                out_tiles[(q, k)] = out_pool.tile(
                    [128, OC], bf16, tag=f"oq{q}", name=f"out_q{q}_k{k}"
                )
                out_fill[(q, k)] = 0
            return out_tiles[(q, k)]

        h_live = {}

        def emit_l2_mm(s, q):
            h = h_live.pop((s, q))
            l1 = l1q[q]
            for j, t in enumerate(QUADS[q]):
                pin("pe", nc.tensor.matmul(
                    l1[32 * j : 32 * j + 8, 0:SLAB],
                    w2c_sb[:, 8 * t : 8 * t + 8],
                    h[:, SLAB * j : SLAB * (j + 1)],
                    start=True,
                    stop=True,
                    tile_position=(0, 32 * j),
                    skip_group_check=True,
                ))

        def emit_evac(s, q):
            l1 = l1q[q]
            k, oo = divmod(s * SLAB, OC)
            ot = out_tile(q, k)
            pin("dve", nc.vector.tensor_scalar_add(
                ot[:, oo : oo + SLAB], l1[:, 0:SLAB], b2_sb[:, q : q + 1]
            ))
            out_fill[(q, k)] += 1
            if k == BC // OC - 1:
                # Final chunk: DMA per slab so the kernel tail only
                # waits on one small last transfer.  The very last DMA
                # (all ACTIVATEs retired by then) goes on the idle
                # scalar HWDGE queue -- gpsimd's SWDGE adds ~1us per
                # issue and a multi-us tail drain.
                lo = (out_fill[(q, k)] - 1) * SLAB
                if s == NSLAB - 1 and q == 1:
                    qk = ("act", nc.scalar)
                elif q == 0:
                    qk = ("sync", nc.sync)
                else:
                    qk = ("gp", nc.gpsimd)
                pin(qk[0], qk[1].dma_start(
                    out_d[q].ap()[:, k * OC + lo : k * OC + lo + SLAB],
                    ot[:, lo : lo + SLAB],
                ))
                if out_fill[(q, k)] == OC // SLAB:
                    del out_tiles[(q, k)]
            elif out_fill[(q, k)] == OC // SLAB:
                # Only partitions 32j..32j+8 carry mus; DMA just those
                # strips (4x fewer HBM bytes than the full panel).
                qk = ("sync", nc.sync) if q == 0 else ("gp", nc.gpsimd)
                for j in range(4):
                    pin(qk[0], qk[1].dma_start(
                        out_d[q].ap()[32 * j : 32 * j + 8,
                                      k * OC : (k + 1) * OC],
                        ot[32 * j : 32 * j + 8, :],
                    ))
                del out_tiles[(q, k)]

        def l1_fill(s, q, j):
            c = s * SLAB
            pin("pe", nc.tensor.matmul(
                l1q[q][:, SLAB * j : SLAB * (j + 1)],
                w1b_sb[32 * j : 32 * j + 16, 128 * q : 128 * (q + 1)],
                xb_sb[q][32 * j : 32 * j + 16, c : c + SLAB],
                start=True,
                stop=True,
                tile_position=(32 * j, 0),
            ))

        for s in range(NSLAB):
            for q in range(2):
                # Per-phase order: L2(s-1), evac(s-1), refill j=1..3,0,
                # tanh.  Dep tracking is whole-tile, so the refill waits
                # on the evac regardless of j order; the serial chain
                # ACT->L2->evac->refill paces the pipeline at the cold
                # 1.2 GHz PE clock.  The dummy LDWEIGHTS below keep the
                # PE activity window hot so DVFS lifts PE to 2.4 GHz,
                # shrinking the chain below the ACT slot.
                if s > 0:
                    emit_l2_mm(s - 1, q)
                    emit_evac(s - 1, q)
                for j in (1, 2, 3, 0):
                    l1_fill(s, q, j)
                h = h_pool.tile([128, QW], bf16, tag="h")
                pin("act", nc.scalar.activation(
                    h[:], l1q[q][:], mybir.ActivationFunctionType.Tanh))
                h_live[(s, q)] = h
                nspam = 2 if s < 2 else 6
                for _ in range(nspam):
                    pin("pe", nc.tensor.ldweights(
                        w1b_sb[0:16, 0:128], tile_position=(0, 0)))
        for q in range(2):
            emit_l2_mm(NSLAB - 1, q)
            emit_evac(NSLAB - 1, q)

    nc.finalize()
    return nc


def _get_nc():
    if "nc" not in _COMPILED:
        _COMPILED["nc"] = _build_nc()
    return _COMPILED["nc"]


def kernel(gt_labels, W1, b1, W2, b2, parent_idx):
    from concourse.bass_utils import run_bass_kernel_spmd

    gt_labels = np.asarray(gt_labels, np.float32)
    w1b_h, w2c_h, b2r = _build_weights(W1, b1, W2, b2, parent_idx)

    in_maps = []
    for c in range(NCORES):
        xb = _build_bands(gt_labels[c * BC : (c + 1) * BC])
        in_maps.append({"xb0": xb[0], "xb1": xb[1], "w1b": w1b_h,
                        "w2c": w2c_h, "b2": b2r})

    nc = _get_nc()
    trace = bool(int(os.environ.get("KERNEL_TRACE", "0")))
    res = run_bass_kernel_spmd(nc, in_maps, list(range(NCORES)), trace=trace)
    if trace and res.exec_time_ns is not None:
        print(f"HW exec time: {res.exec_time_ns} ns")
        _COMPILED["exec_time_ns"] = res.exec_time_ns

    mus = np.empty((BATCH, NUM_NODES), np.float32)
    for c in range(NCORES):
        rows = []
        for q in range(2):
            panel = np.asarray(res.results[c][f"out{q}"], np.float32)
            for j in range(4):
                rows.append(panel[32 * j : 32 * j + 8])  # nodes 32q+8j..+8
        mus[c * BC : (c + 1) * BC] = np.concatenate(rows, axis=0).T
    mus = mus.reshape(BATCH, NUM_NODES, 1)
    logvars = np.zeros_like(mus)
    return mus, logvars

